# revision 6
# baseline (speedup 1.0000x reference)
"""Trainium2 Bass kernel for the Phase2Z VQ-codebook model.

Reference computation (see problem):
  1. latent positions  = sorted columns where input_ids == LATENT_ID (3)
  2. inputs_embeds     = emb_weight[input_ids]                       [B,T,H]
  3. z_ids             = argmin_z cdist(l2norm(latent_states), l2norm(Ez))
                         (== argmax_z  dot(latent, Ez_z) / ||Ez_z||)
  4. inputs_embeds[b, latent_pos[b,k]] = Ez[z_ids[b,k]]  (active z_mask only)
  5. digit_logits      = einsum('bh,dhc->bdc', embeds[b, ans_idx], head_w) + head_b

Device strategy (8 NeuronCores, data-parallel over batch, 2 rows/core):
  - Stream A: 16x indirect-DMA gather of emb rows by token id -> SBUF tile
    [128, H] -> indirect-DMA scatter to the output (row index list), with
    active-latent rows skipped via an out-of-bounds sentinel + bounds_check.
  - VQ (overlapped with stream A): S = latentT.T @ EzT on PE (f32), column
    norms of Ez via ACT Square + ones-vector matmul, scale via rsqrt +
    broadcast matmul, then DVE max/max_index -> z ids; indirect gather of
    the chosen codebook rows and indirect scatter into the latent positions
    (disjoint from stream A's rows, so no write ordering hazard).
  - Head: h = emb[ans_token_id] (ans position is never an active latent slot
    in this model: the ANSWER token id is 2, latent slots require id 3);
    PE-transpose h, 16 accumulating matmuls against host-pre-swizzled
    head weights, per-partition bias add, emitted transposed [120, B_local].

Host only marshals layouts/index lists (numpy); all FLOPs and all memory
movement of tensor data run on the NeuronCores.
"""

import os
import numpy as np

B, T, H = 16, 1024, 2048
KMAX, ZV = 64, 512
NDIG, NCLS = 12, 10
VOCAB = 32000
ANSWER_ID, LATENT_ID, Z_START = 2, 3, 1000

NCORES = 8
BPC = B // NCORES          # batch rows per core
TOK = BPC * T              # tokens per core
NT = TOK // 128            # gather tiles per core
BK = BPC * KMAX            # latent slots per core (=128)
HC = H // 128              # 128-wide h chunks
DC = NDIG * NCLS           # 120
SENT = 1 << 20             # out-of-bounds sentinel row index (skipped)

_CACHE: dict = {}
LAST_EXEC_NS = None
LAST_RESULTS = None


def _build_program():
    import concourse.bacc as bacc
    import concourse.bass as bass
    import concourse.mybir as mybir
    import concourse.tile as tile
    from concourse.masks import make_identity

    f32 = mybir.dt.float32
    i32 = mybir.dt.int32
    u32 = mybir.dt.uint32
    AF = mybir.ActivationFunctionType

    nc = bacc.Bacc("TRN2", target_bir_lowering=False)

    emb = nc.dram_tensor("emb", [VOCAB, H], f32, kind="ExternalInput")
    ezt = nc.dram_tensor("ezt", [128, HC * ZV], f32, kind="ExternalInput")
    ltt = nc.dram_tensor("ltt", [128, HC * 128], f32, kind="ExternalInput")
    w2 = nc.dram_tensor("w2", [128, HC * DC], f32, kind="ExternalInput")
    hbt = nc.dram_tensor("hbt", [DC, 1], f32, kind="ExternalInput")
    # meta columns: [0:NT] gather token ids, [NT:2NT] scatter row ids,
    # [2NT] latent scatter row ids, [2NT+1] answer token ids (rows 0..BPC-1)
    meta = nc.dram_tensor("meta", [128, 2 * NT + 2], i32, kind="ExternalInput")

    out_e = nc.dram_tensor("out_e", [TOK, H], f32, kind="ExternalOutput")
    out_d = nc.dram_tensor("out_d", [DC, BPC], f32, kind="ExternalOutput")

    with tile.TileContext(nc) as tc:
        with (
            tc.tile_pool(name="const", bufs=1) as constp,
            tc.tile_pool(name="big", bufs=1) as bigp,
            tc.tile_pool(name="ga", bufs=4) as gap,
            tc.tile_pool(name="sq", bufs=2) as sqp,
            tc.tile_pool(name="ps", bufs=1, space="PSUM") as psp,
            tc.tile_pool(name="pst", bufs=2, space="PSUM") as pstp,
        ):
            meta_sb = constp.tile([128, 2 * NT + 2], i32)
            nc.sync.dma_start(meta_sb[:], meta[:])

            # ---- stream A: bulk token gather/scatter ----
            for c in range(NT):
                t = gap.tile([128, H], f32, tag="ga")
                nc.gpsimd.indirect_dma_start(
                    out=t[:],
                    out_offset=None,
                    in_=emb[:, :],
                    in_offset=bass.IndirectOffsetOnAxis(
                        ap=meta_sb[:, c:c + 1], axis=0
                    ),
                )
                nc.gpsimd.indirect_dma_start(
                    out=out_e[:, :],
                    out_offset=bass.IndirectOffsetOnAxis(
                        ap=meta_sb[:, NT + c:NT + c + 1], axis=0
                    ),
                    in_=t[:],
                    in_offset=None,
                    bounds_check=TOK - 1,
                    oob_is_err=False,
                )

            # ---- VQ: similarity matmul + codebook norms + argmax ----
            ezt_sb = bigp.tile([128, HC * ZV], f32)
            nc.sync.dma_start(ezt_sb[:], ezt[:])
            ltt_sb = bigp.tile([128, HC * 128], f32)
            nc.sync.dma_start(ltt_sb[:], ltt[:])

            ones_col = constp.tile([128, 1], f32)
            nc.vector.memset(ones_col[:], 1.0)
            ones_row = constp.tile([1, 128], f32)
            nc.vector.memset(ones_row[:], 1.0)

            norms2_ps = psp.tile([1, ZV], f32, space="PSUM")
            for c in range(HC):
                sq = sqp.tile([128, ZV], f32, tag="sq")
                nc.scalar.activation(
                    sq[:], ezt_sb[:, c * ZV:(c + 1) * ZV], AF.Square
                )
                nc.tensor.matmul(
                    out=norms2_ps[:], lhsT=ones_col[:], rhs=sq[:],
                    start=(c == 0), stop=(c == HC - 1),
                )
            nrm = constp.tile([1, ZV], f32)
            nc.scalar.activation(nrm[:], norms2_ps[:], AF.Sqrt)
            rnorm = constp.tile([1, ZV], f32)
            nc.vector.reciprocal(rnorm[:], nrm[:])
            bcast_ps = psp.tile([128, ZV], f32, space="PSUM")
            nc.tensor.matmul(
                out=bcast_ps[:], lhsT=ones_row[:], rhs=rnorm[:],
                start=True, stop=True,
            )

            s_ps = psp.tile([128, ZV], f32, space="PSUM")
            for c in range(HC):
                nc.tensor.matmul(
                    out=s_ps[:],
                    lhsT=ltt_sb[:, c * 128:(c + 1) * 128],
                    rhs=ezt_sb[:, c * ZV:(c + 1) * ZV],
                    start=(c == 0), stop=(c == HC - 1),
                )
            bcast_sb = constp.tile([128, ZV], f32)
            nc.scalar.copy(bcast_sb[:], bcast_ps[:])
            s_sb = constp.tile([128, ZV], f32)
            nc.vector.tensor_tensor(
                out=s_sb[:], in0=s_ps[:], in1=bcast_sb[:],
                op=mybir.AluOpType.mult,
            )

            max8 = constp.tile([128, 8], f32)
            idx8 = constp.tile([128, 8], u32)
            nc.vector.max(max8[:], s_sb[:])
            nc.vector.max_index(idx8[:], max8[:], s_sb[:])

            zrows = bigp.tile([128, H], f32)
            nc.gpsimd.indirect_dma_start(
                out=zrows[:],
                out_offset=None,
                in_=emb[:, :],
                in_offset=bass.IndirectOffsetOnAxis(ap=idx8[:, :1], axis=0),
                element_offset=Z_START * H,
            )
            nc.gpsimd.indirect_dma_start(
                out=out_e[:, :],
                out_offset=bass.IndirectOffsetOnAxis(
                    ap=meta_sb[:, 2 * NT:2 * NT + 1], axis=0
                ),
                in_=zrows[:],
                in_offset=None,
                bounds_check=TOK - 1,
                oob_is_err=False,
            )

            # ---- digit head at the answer position ----
            w2_sb = bigp.tile([128, HC * DC], f32)
            nc.sync.dma_start(w2_sb[:], w2[:])
            hbt_sb = constp.tile([DC, 1], f32)
            nc.sync.dma_start(hbt_sb[:], hbt[:])

            h_sb = constp.tile([BPC, H], f32)
            nc.gpsimd.indirect_dma_start(
                out=h_sb[:],
                out_offset=None,
                in_=emb[:, :],
                in_offset=bass.IndirectOffsetOnAxis(
                    ap=meta_sb[0:BPC, 2 * NT + 1:2 * NT + 2], axis=0
                ),
            )
            ident = constp.tile([128, 128], f32)
            make_identity(nc, ident[:])

            ht_sb = constp.tile([128, HC * BPC], f32)
            for c in range(HC):
                tp = pstp.tile([128, BPC], f32, tag="tp", space="PSUM")
                nc.tensor.transpose(
                    tp[:], h_sb[:, c * 128:(c + 1) * 128], ident[:BPC, :BPC]
                )
                nc.vector.tensor_copy(
                    ht_sb[:, c * BPC:(c + 1) * BPC], tp[:]
                )

            logits_ps = psp.tile([DC, BPC], f32, space="PSUM")
            for c in range(HC):
                nc.tensor.matmul(
                    out=logits_ps[:],
                    lhsT=w2_sb[:, c * DC:(c + 1) * DC],
                    rhs=ht_sb[:, c * BPC:(c + 1) * BPC],
                    start=(c == 0), stop=(c == HC - 1),
                )
            logt_sb = constp.tile([DC, BPC], f32)
            nc.vector.tensor_scalar_add(logt_sb[:], logits_ps[:], hbt_sb[:, :1])
            nc.sync.dma_start(out_d[:], logt_sb[:])

    nc.compile()
    return nc


def _swizzle(a, p=128):
    """[C*p, F] -> [p, C*F] so that chunk c of the SBUF tile holds rows
    c*p..c*p+p-1 of `a` (partition = row within chunk)."""
    cp, f = a.shape
    c = cp // p
    return np.ascontiguousarray(
        a.reshape(c, p, f).transpose(1, 0, 2).reshape(p, c * f)
    )


def _prepare_in_maps(input_ids, latent_states, z_mask,
                     emb_weight, head_w, head_b):
    ids = np.asarray(input_ids).astype(np.int32)           # [B, T]
    latent_states = np.ascontiguousarray(
        np.asarray(latent_states, dtype=np.float32))       # [B, KMAX, H]
    zm = np.asarray(z_mask).astype(bool)                   # [B, KMAX]
    emb_np = np.ascontiguousarray(np.asarray(emb_weight, dtype=np.float32))
    head_w_np = np.asarray(head_w, dtype=np.float32)       # [NDIG, H, NCLS]
    head_b_np = np.asarray(head_b, dtype=np.float32)       # [NDIG, NCLS]

    # ---- host index marshalling (mirrors the reference's vectorized
    #      _infer_latent_positions exactly) ----
    col = np.arange(T)
    is_lat = ids == LATENT_ID
    order = np.argsort(np.where(is_lat, col, col + T), axis=1, kind="stable")
    latent_pos = order[:, :KMAX]                           # [B, KMAX]
    ans_idx = np.argmax(ids == ANSWER_ID, axis=1)          # [B]
    ans_tok = ids[np.arange(B), ans_idx]                   # [B]

    # replicated device inputs
    ez_t = _swizzle(np.ascontiguousarray(
        emb_np[Z_START:Z_START + ZV].T))                   # [128, 16*512]
    w2_np = _swizzle(np.ascontiguousarray(
        head_w_np.transpose(1, 0, 2).reshape(H, DC)))      # [128, 16*120]
    hbt_np = np.ascontiguousarray(head_b_np.reshape(DC, 1))

    in_maps = []
    for core in range(NCORES):
        b0 = core * BPC
        ids_flat = ids[b0:b0 + BPC].reshape(TOK)
        gids = ids_flat.reshape(NT, 128).T                 # [128, NT]

        pos_flat = np.arange(TOK, dtype=np.int32)
        skip = np.zeros(TOK, dtype=bool)
        lat_rows = np.full((BPC, KMAX), SENT, dtype=np.int32)
        for bl in range(BPC):
            b = b0 + bl
            act = zm[b]
            rows = bl * T + latent_pos[b][act]
            skip[rows] = True
            lat_rows[bl][act] = rows.astype(np.int32)
        outpos = np.where(skip, SENT, pos_flat).reshape(NT, 128).T

        ans_col = np.zeros(128, dtype=np.int32)
        ans_col[:BPC] = ans_tok[b0:b0 + BPC]

        meta = np.concatenate(
            [gids, outpos, lat_rows.reshape(BK, 1), ans_col.reshape(128, 1)],
            axis=1).astype(np.int32)
        meta = np.ascontiguousarray(meta)                  # [128, 2*NT+2]

        ltt = _swizzle(np.ascontiguousarray(
            latent_states[b0:b0 + BPC].reshape(BK, H).T))  # [128, 16*128]

        in_maps.append({
            "emb": emb_np,
            "ezt": ez_t,
            "ltt": ltt,
            "w2": w2_np,
            "hbt": hbt_np,
            "meta": meta,
        })
    return in_maps


def _get_nc():
    if "nc" not in _CACHE:
        _CACHE["nc"] = _build_program()
    return _CACHE["nc"]


def kernel(input_ids, attention_mask, latent_states, z_mask,
           emb_weight, head_w, head_b):
    global LAST_EXEC_NS, LAST_RESULTS
    from concourse import bass_utils

    in_maps = _prepare_in_maps(input_ids, latent_states, z_mask,
                               emb_weight, head_w, head_b)
    nc = _get_nc()

    res = bass_utils.run_bass_kernel_spmd(
        nc, in_maps, core_ids=list(range(NCORES)),
        trace=bool(int(os.environ.get("KERNEL_TRACE", "0"))),
    )
    LAST_RESULTS = res
    LAST_EXEC_NS = res.exec_time_ns

    embeds = np.concatenate(
        [r["out_e"].reshape(BPC, T, H) for r in res.results], axis=0)
    logits = np.concatenate(
        [np.ascontiguousarray(r["out_d"].T).reshape(BPC, NDIG, NCLS)
         for r in res.results], axis=0)
    return logits, embeds
